# revision 1
# baseline (speedup 1.0000x reference)
"""Distributed multi-head attention block on 8 TRN2 NeuronCores.

Problem: B=4, S=2048, E=1024, H=16 heads, D=64.
Sharding: core c -> (batch b = c//2, head-group g = c%2 of 8 heads).
Per core: QKV projections for its 512 n-dims, attention for 8 heads,
AllGather of attention outputs within each core pair, out-projection of
a disjoint 512-column slice of the output. Everything transposed
("orientation-2"): q^T/k^T/v computed directly in the layouts the PE
matmuls want, so no on-device transposes are needed.

Compute dtype: bf16 on the TensorEngine (f32 PSUM accumulation), exp on
ScalarE from f32 scores. Softmax denominators come from a ones-column
appended to V; 1/denom via exp(-ln(denom)) on ScalarE; broadcast of the
per-column reciprocal across partitions via a rank-1 PE matmul.
"""

import os
import sys

sys.path.insert(0, "/opt/trn_rl_repo")

import numpy as np

import concourse.bass as bass
import concourse.bacc as bacc
import concourse.mybir as mybir
import concourse.tile as tile
from concourse.bass_utils import run_bass_kernel_spmd

bf16 = mybir.dt.bfloat16
f32 = mybir.dt.float32
f32r = mybir.dt.float32r
AF = mybir.ActivationFunctionType

N_CORES = 8

# Full problem dims
B, S, E, H, D = 4, 2048, 1024, 16, 64
G = 2            # head-groups (tensor-parallel degree within a batch)
NS = E // G      # 512: n-dims (head dims) per core
HL = H // G      # 8 heads per core
EC = E // 128    # 8 contraction chunks for projections
NT = NS // 128   # 4 tiles of q^T/k^T (= head pairs)
TT = S // 128    # 16 t-tiles
SCW = 512        # s-chunk width
SC = S // SCW    # 4 s-chunks
SCALE = 1.0 / np.sqrt(D)

REPLICA_GROUPS = [[2 * i, 2 * i + 1] for i in range(4)]

_CACHE = {}


def build(odd_evict_via_gpsimd=False, debug=False, taps=False):
    """Build the SPMD bass graph (identical on all 8 cores)."""
    nc = bacc.Bacc("TRN2", target_bir_lowering=False, debug=debug,
                   num_devices=N_CORES)

    # --- per-core external I/O (shards prepared host-side) ---
    x_ext = nc.dram_tensor("xT", [EC, 128, S], bf16, kind="ExternalInput")
    wq_ext = nc.dram_tensor("wq", [EC, 128, NS], bf16, kind="ExternalInput")
    wk_ext = nc.dram_tensor("wk", [EC, 128, NS], bf16, kind="ExternalInput")
    wv_ext = nc.dram_tensor("wv", [EC, 128, NS], bf16, kind="ExternalInput")
    wo_ext = nc.dram_tensor("wo", [EC, 128, NS], bf16, kind="ExternalInput")
    bq_ext = nc.dram_tensor("bq", [128, NT], f32, kind="ExternalInput")
    bk_ext = nc.dram_tensor("bk", [128, NT], f32, kind="ExternalInput")
    bv_ext = nc.dram_tensor("bv", [128, NT], f32, kind="ExternalInput")
    bo_ext = nc.dram_tensor("bo", [128, NT], f32, kind="ExternalInput")
    out_ext = nc.dram_tensor("out", [NT, 128, S], f32, kind="ExternalOutput")

    with tile.TileContext(nc) as tc:
        with (
            tc.tile_pool(name="big", bufs=1) as big,        # persistent sbuf
            tc.tile_pool(name="et", bufs=4) as etp,         # E^T double+ buffers
            tc.tile_pool(name="small", bufs=2) as small,    # rows / tmp
            tc.tile_pool(name="ps_sc", bufs=4, space="PSUM") as ps_sc,
            tc.tile_pool(name="ps_un", bufs=2, space="PSUM") as ps_un,
            tc.tile_pool(name="dram", bufs=1, space="DRAM") as dram,
        ):
            # ---- persistent SBUF tensors ----
            xT = big.tile([128, EC, S], bf16)                  # x^T  (e, s)
            wq = big.tile([128, EC, NS], bf16)
            wk = big.tile([128, EC, NS], bf16)
            wv = big.tile([128, EC, NS], bf16)
            wo = big.tile([128, EC, NS], bf16)
            bq_sb = big.tile([128, NT], f32)
            bk_sb = big.tile([128, NT], f32)
            bv_sb = big.tile([128, NT], f32)
            bo_sb = big.tile([128, NT], f32)
            qT = big.tile([128, NT, S], bf16)                  # q^T  (n, s)
            kT = big.tile([128, NT, S], bf16)                  # k^T  (n, t)
            v_sb = big.tile([128, TT, HL, D + 1], bf16)        # v (+ones col)
            aT = big.tile([128, NT, S], bf16)                  # attn out^T (n, s)
            # gathered attn out^T, global-interleaved chunk order
            # [0,1,4,5, 2,3,6,7]; reuses xT's slot (same tag, bufs=1)
            aT_full = big.tile([128, EC, S], bf16, tag="xT")
            ones_sb = big.tile([1, D], bf16)

            for c in range(EC):
                nc.sync.dma_start(xT[:, c, :], x_ext[c])
            nc.sync.dma_start(wq[:], wq_ext[:].rearrange("c p n -> p c n"))
            nc.sync.dma_start(wk[:], wk_ext[:].rearrange("c p n -> p c n"))
            nc.sync.dma_start(wv[:], wv_ext[:].rearrange("c p n -> p c n"))
            nc.sync.dma_start(wo[:], wo_ext[:].rearrange("c p n -> p c n"))
            nc.sync.dma_start(bq_sb[:], bq_ext[:])
            nc.sync.dma_start(bk_sb[:], bk_ext[:])
            nc.sync.dma_start(bv_sb[:], bv_ext[:])
            nc.sync.dma_start(bo_sb[:], bo_ext[:])
            nc.vector.memset(ones_sb[:], 1.0)
            nc.vector.memset(v_sb[:, :, :, D:D + 1], 1.0)

            # ---- q^T / k^T projections:  out[n_tile, s] = W^T.T @ x^T ----
            # ec-major inner pair so the stationary operand is reused
            # across s-chunks
            for w_sb, b_sb, dst in ((wq, bq_sb, qT), (wk, bk_sb, kT)):
                for nt in range(NT):
                    pss = [ps_sc.tile([128, SCW], f32, tag="bank", name="psq")
                           for _ in range(SC)]
                    for ec in range(EC):
                        for sc in range(SC):
                            nc.tensor.matmul(
                                pss[sc][:],
                                w_sb[:, ec, 128 * nt:128 * (nt + 1)],
                                xT[:, ec, SCW * sc:SCW * (sc + 1)],
                                start=(ec == 0), stop=(ec == EC - 1),
                            )
                    for sc in range(SC):
                        nc.vector.tensor_scalar_add(
                            dst[:, nt, SCW * sc:SCW * (sc + 1)], pss[sc][:],
                            b_sb[:, nt:nt + 1],
                        )

            # ---- v projection: v[t_tile, n] = x^T.T @ Wv^T (bias deferred) ----
            for tt in range(TT):
                ps = ps_sc.tile([128, NS], f32, tag="bank")
                for ec in range(EC):
                    nc.tensor.matmul(
                        ps[:],
                        xT[:, ec, 128 * tt:128 * (tt + 1)],
                        wv[:, ec, :],
                        start=(ec == 0), stop=(ec == EC - 1),
                    )
                # scatter heads into the (D+1)-strided v layout
                nc.vector.tensor_copy(
                    v_sb[:, tt, :, 0:D],
                    ps[:].rearrange("p (h d) -> p h d", h=HL),
                )

            # ---- attention, software-pipelined over (pair, s-chunk) ----
            iters = [(p, sc) for p in range(NT) for sc in range(SC)]

            def emit_scores(p, sc):
                """scores^T -> exp -> E^T tiles for both heads of the pair."""
                ssl = slice(SCW * sc, SCW * (sc + 1))
                ets = []
                for o in (0, 64):  # head 2p (o=0) and 2p+1 (o=64)
                    et = etp.tile([128, TT, SCW], bf16, tag="et")
                    ets.append(et)
                    for tt in range(TT):
                        ps = ps_sc.tile([128, SCW], f32, tag="bank")
                        nc.tensor.matmul(
                            ps[:],
                            kT[o:o + 64, p, 128 * tt:128 * (tt + 1)],
                            qT[o:o + 64, p, ssl],
                            start=True, stop=True,
                        )
                        nc.scalar.activation(et[:, tt, :], ps[:], AF.Exp,
                                             scale=float(SCALE))
                return ets

            def emit_attnv(p, sc, ets):
                """attn @ v, numerator evict + denominator row copy."""
                un = ps_un.tile([128, 2, SCW], f32, tag="un")
                for i in range(2):
                    h = 2 * p + i
                    for tt in range(TT):
                        nc.tensor.matmul(
                            un[0:D + 1, i, :],
                            v_sb[:, tt, h, :],
                            ets[i][:, tt, :],
                            start=(tt == 0), stop=(tt == TT - 1),
                        )
                un_sb = small.tile([128, SCW], f32, tag="un_sb")
                nc.vector.tensor_copy(un_sb[0:D, :], un[0:D, 0, :])
                nc.vector.tensor_copy(un_sb[64:64 + D, :], un[0:D, 1, :])
                denom = small.tile([1, 2, SCW], bf16, tag="denom")
                nc.vector.tensor_copy(denom[:], un[D:D + 1, :, :])
                return un_sb, denom

            def emit_norm(p, sc, un_sb, denom):
                """broadcast denominators, reciprocal, multiply, bias -> aT."""
                ssl = slice(SCW * sc, SCW * (sc + 1))
                bcast = ps_sc.tile([128, SCW], f32, tag="bank")
                nc.tensor.matmul(bcast[0:D, :], ones_sb[:], denom[0:1, 0, :],
                                 start=True, stop=True)
                nc.tensor.matmul(bcast[64:64 + D, :], ones_sb[:],
                                 denom[0:1, 1, :], start=True, stop=True)
                rb = small.tile([128, SCW], f32, tag="rb", bufs=1)
                nc.vector.reciprocal(rb[:], bcast[:])
                nc.vector.tensor_mul(aT[:, p, ssl], un_sb[:], rb[:])
                nc.vector.tensor_scalar_add(aT[:, p, ssl], aT[:, p, ssl],
                                            bv_sb[:, p:p + 1])

            def emit_gather(half):
                cc_in = dram.tile([128, 2, S], bf16, name=f"cc_in{half}")
                cc_out = dram.tile([2, 128, 2, S], bf16, name=f"cc_out{half}")
                nc.sync.dma_start(cc_in[:], aT[:, 2 * half:2 * half + 2, :])
                nc.gpsimd.collective_compute(
                    "AllGather",
                    mybir.AluOpType.bypass,
                    replica_groups=REPLICA_GROUPS,
                    ins=[cc_in[:].opt()],
                    outs=[cc_out[:].opt()],
                )
                for r in range(2):
                    nc.sync.dma_start(
                        aT_full[:, 4 * r + 2 * half:4 * r + 2 * half + 2, :],
                        cc_out[r],
                    )

            prev = None       # (p, sc, ets)
            pend_norm = None  # (p, sc, un_sb, denom)
            for it in iters + [None]:
                if prev is not None:
                    pend_av = emit_attnv(*prev)
                else:
                    pend_av = None
                if it is not None:
                    cur_ets = emit_scores(*it)
                if pend_av is not None:
                    emit_norm(prev[0], prev[1], *pend_av)
                    if prev[1] == SC - 1 and prev[0] in (1, NT - 1):
                        emit_gather(0 if prev[0] == 1 else 1)
                prev = (it[0], it[1], cur_ets) if it is not None else None

            # ---- out-projection: o^T[e_tile, s] = Wo^T.T @ aT_full ----
            for et_i in range(NT):
                for scp in range(SC // 2):
                    po = ps_un.tile([128, 2, SCW], f32, tag="un")
                    for scq in range(2):
                        sc = 2 * scp + scq
                        for cchunk in range(EC):
                            nc.tensor.matmul(
                                po[:, scq, :],
                                wo[:, cchunk, 128 * et_i:128 * (et_i + 1)],
                                aT_full[:, cchunk, SCW * sc:SCW * (sc + 1)],
                                start=(cchunk == 0), stop=(cchunk == EC - 1),
                            )
                        o_st = small.tile([128, SCW], f32, tag="o_st", bufs=1)
                        nc.vector.tensor_scalar_add(
                            o_st[:], po[:, scq, :], bo_sb[:, et_i:et_i + 1])
                        nc.sync.dma_start(
                            out_ext[et_i, :, SCW * sc:SCW * (sc + 1)],
                            o_st[:],
                        )

            if taps:
                for nm, t in (("qT", qT), ("kT", kT), ("v_sb", v_sb),
                              ("aT", aT), ("aT_full", aT_full)):
                    ext = nc.dram_tensor(f"dbg_{nm}", list(t.shape), t.dtype,
                                         kind="ExternalOutput")
                    nc.sync.dma_start(ext[:], t[:])
    nc.compile()
    return nc


def _prep_inputs(x, Wq, bq, Wk, bk, Wv, bv, Wo, bo):
    """Shard + lay out the full inputs for the 8 cores."""
    import ml_dtypes
    bfl = ml_dtypes.bfloat16

    # aT_full lands in plain global n-chunk order (position 4r+2*half+j)
    wo_order = [0, 1, 2, 3, 4, 5, 6, 7]

    in_maps = []
    for c in range(N_CORES):
        b, g = divmod(c, G)
        ns = slice(NS * g, NS * (g + 1))
        xT = np.ascontiguousarray(x[b].T).astype(bfl).reshape(EC, 128, S)
        wq_l = np.ascontiguousarray(Wq[ns, :].T).astype(bfl).reshape(EC, 128, NS)
        wk_l = np.ascontiguousarray(Wk[ns, :].T).astype(bfl).reshape(EC, 128, NS)
        wv_l = np.ascontiguousarray(Wv[ns, :].T).astype(bfl).reshape(EC, 128, NS)
        # Wo^T rows (contraction n) in gathered order, cols = this core's
        # e-slice
        woT = np.ascontiguousarray(Wo[ns, :].T)  # [E, NS] = Wo.T[:, es]
        woT = woT.reshape(EC, 128, NS)[wo_order]
        wo_l = woT.astype(bfl).reshape(EC, 128, NS)
        bq_l = np.ascontiguousarray(bq[ns].reshape(NT, 128).T).astype(np.float32)
        bk_l = np.ascontiguousarray(bk[ns].reshape(NT, 128).T).astype(np.float32)
        bv_l = np.ascontiguousarray(bv[ns].reshape(NT, 128).T).astype(np.float32)
        bo_l = np.ascontiguousarray(bo[ns].reshape(NT, 128).T).astype(np.float32)
        in_maps.append({
            "xT": np.ascontiguousarray(xT),
            "wq": np.ascontiguousarray(wq_l),
            "wk": np.ascontiguousarray(wk_l),
            "wv": np.ascontiguousarray(wv_l),
            "wo": np.ascontiguousarray(wo_l),
            "bq": bq_l, "bk": bk_l, "bv": bv_l, "bo": bo_l,
        })
    return in_maps


def kernel(x, Wq, bq, Wk, bk, Wv, bv, Wo, bo, _trace=False):
    x = np.asarray(x)
    in_maps = _prep_inputs(np.asarray(x, np.float32),
                           *[np.asarray(a, np.float32)
                             for a in (Wq, bq, Wk, bk, Wv, bv, Wo, bo)])
    if "nc" not in _CACHE:
        _CACHE["nc"] = build()
    nc = _CACHE["nc"]
    res = run_bass_kernel_spmd(nc, in_maps, core_ids=list(range(N_CORES)),
                               trace=_trace)
    _CACHE["last_result"] = res

    out = np.empty((B, S, E), np.float32)
    for c in range(N_CORES):
        b, g = divmod(c, G)
        oT = res.results[c]["out"].reshape(NS, S)  # [e_sub, s]
        out[b, :, NS * g:NS * (g + 1)] = oT.T
    return out


if __name__ == "__main__":
    nc = build()
    print("built ok:", len(nc.inst_map), "instructions")



# revision 6
# speedup vs baseline: 1.2925x; 1.2925x over previous
"""Distributed multi-head attention block on 8 TRN2 NeuronCores.

Problem: B=4, S=2048, E=1024, H=16 heads, D=64.
Sharding: core c -> (batch b = c//2, head-group g = c%2 of 8 heads).

v2 pipeline: one global software pipeline. Attention is ScalarE(exp)-
bound, so all projection matmuls (full 128x128 array utilization) are
injected into the PE idle slots of the attention loop. This keeps the
PE HAM activity monitor at K=8/8 (2.4 GHz) -- the v1 kernel ran the
whole attention phase at K=4/8 because its 64-row/65-col matmuls with
idle gaps read as ~50% utilization.

Scores h0/h64 are issued adjacently so the two 64-row matmuls execute
concurrently in separate PE row-groups. Exp reads 2 PSUM banks per
ACTIVATE (N=1024) to amortize the 352-cycle ACT overhead. Softmax
denominators ride the ones-column appended to V; 1/denom via the fast
DVE reciprocal on the compact [1,2,512] row, then broadcast via rank-1
PE matmuls.
"""

import os
import sys

sys.path.insert(0, "/opt/trn_rl_repo")

import numpy as np

import concourse.bass as bass
import concourse.bacc as bacc
import concourse.mybir as mybir
import concourse.tile as tile
from concourse.bass_utils import run_bass_kernel_spmd

bf16 = mybir.dt.bfloat16
f32 = mybir.dt.float32
AF = mybir.ActivationFunctionType

N_CORES = 8

# Full problem dims
B, S, E, H, D = 4, 2048, 1024, 16, 64
G = 2            # head-groups (tensor-parallel degree within a batch)
NS = E // G      # 512: n-dims (head dims) per core
HL = H // G      # 8 heads per core
EC = E // 128    # 8 contraction chunks for projections
NT = NS // 128   # 4 tiles of q^T/k^T (= head pairs)
TT = S // 128    # 16 t-tiles
SCW = 512        # s-chunk width
SC = S // SCW    # 4 s-chunks
NG = TT // 2     # 8 tt-groups of 2 per attention iteration
SCALE = 1.0 / np.sqrt(D)

REPLICA_GROUPS = [[2 * i, 2 * i + 1] for i in range(4)]

# aT_full chunk order (gather writes row r of half h to slot 4r+2h)
CC_ORDER = [0, 1, 4, 5, 2, 3, 6, 7]  # gather-0 chunks first

_CACHE = {}


def build(debug=False):
    """Build the SPMD bass graph (identical on all 8 cores)."""
    nc = bacc.Bacc("TRN2", target_bir_lowering=False, debug=debug,
                   num_devices=N_CORES)

    # --- per-core external I/O (shards prepared host-side) ---
    x_ext = nc.dram_tensor("xT", [EC, 128, S], bf16, kind="ExternalInput")
    wq_ext = nc.dram_tensor("wq", [EC, 128, NS], bf16, kind="ExternalInput")
    wk_ext = nc.dram_tensor("wk", [EC, 128, NS], bf16, kind="ExternalInput")
    wv_ext = nc.dram_tensor("wv", [EC, 128, NS], bf16, kind="ExternalInput")
    wo_ext = nc.dram_tensor("wo", [EC, 128, NS], bf16, kind="ExternalInput")
    bq_ext = nc.dram_tensor("bq", [128, NT], f32, kind="ExternalInput")
    bk_ext = nc.dram_tensor("bk", [128, NT], f32, kind="ExternalInput")
    bv_ext = nc.dram_tensor("bv", [128, NT], f32, kind="ExternalInput")
    bo_ext = nc.dram_tensor("bo", [128, NT], f32, kind="ExternalInput")
    out_ext = nc.dram_tensor("out", [NT, 128, S], f32, kind="ExternalOutput")

    with tile.TileContext(nc) as tc:
        with (
            tc.tile_pool(name="big", bufs=1) as big,        # persistent sbuf
            tc.tile_pool(name="et", bufs=8) as etp,         # E^T tiles
            tc.tile_pool(name="small", bufs=2) as small,    # rows / tmp
            tc.tile_pool(name="ps_s", bufs=2, space="PSUM") as ps_s,    # 4 banks
            tc.tile_pool(name="ps_w", bufs=2, space="PSUM") as ps_w,    # 2 banks
            tc.tile_pool(name="ps_un", bufs=1, space="PSUM") as ps_un,  # 2 banks
            tc.tile_pool(name="dram", bufs=1, space="DRAM") as dram,
        ):
            # ---- persistent SBUF tensors ----
            xT = big.tile([128, EC, S], bf16)                  # x^T  (e, s)
            wq = big.tile([128, EC, NS], bf16)
            wk = big.tile([128, EC, NS], bf16)
            wv = big.tile([128, EC, NS], bf16)
            wo = big.tile([128, EC, NS], bf16)
            bq_sb = big.tile([128, NT], f32)
            bk_sb = big.tile([128, NT], f32)
            bv_sb = big.tile([128, NT], f32)
            bo_sb = big.tile([128, NT], f32)
            qT = big.tile([128, NT, S], bf16)                  # q^T  (n, s)
            kT = big.tile([128, NT, S], bf16)                  # k^T  (n, t)
            v_sb = big.tile([128, TT, HL, D + 1], bf16)        # v (+ones col)
            aT = big.tile([128, NT, S], bf16)                  # attn out^T (n, s)
            aT_full = big.tile([128, EC, S], bf16)             # gathered attn^T
            ones_sb = big.tile([1, D], bf16)

            for c in range(EC):
                nc.sync.dma_start(xT[:, c, :], x_ext[c])
            nc.sync.dma_start(wq[:], wq_ext[:].rearrange("c p n -> p c n"))
            nc.sync.dma_start(wk[:], wk_ext[:].rearrange("c p n -> p c n"))
            nc.sync.dma_start(wv[:], wv_ext[:].rearrange("c p n -> p c n"))
            nc.sync.dma_start(wo[:], wo_ext[:].rearrange("c p n -> p c n"))
            nc.sync.dma_start(bq_sb[:], bq_ext[:])
            nc.sync.dma_start(bk_sb[:], bk_ext[:])
            nc.sync.dma_start(bv_sb[:], bv_ext[:])
            nc.sync.dma_start(bo_sb[:], bo_ext[:])
            nc.vector.memset(ones_sb[:], 1.0)
            nc.vector.memset(v_sb[:, :, :, D:D + 1], 1.0)

            # ---- background projection generators (one yield per MM) ----
            def gen_qk(w_sb, b_sb, dst, nt, sc):
                ssl = slice(SCW * sc, SCW * (sc + 1))
                ps = ps_w.tile([128, SCW], f32, tag="proj")
                for ec in range(EC):
                    nc.tensor.matmul(
                        ps[:],
                        w_sb[:, ec, 128 * nt:128 * (nt + 1)],
                        xT[:, ec, ssl],
                        start=(ec == 0), stop=(ec == EC - 1),
                    )
                    yield
                nc.vector.tensor_scalar_add(dst[:, nt, ssl], ps[:],
                                            b_sb[:, nt:nt + 1])

            def gen_v(tt):
                ps = ps_w.tile([128, NS], f32, tag="proj")
                for ec in range(EC):
                    nc.tensor.matmul(
                        ps[:],
                        xT[:, ec, 128 * tt:128 * (tt + 1)],
                        wv[:, ec, :],
                        start=(ec == 0), stop=(ec == EC - 1),
                    )
                    yield
                nc.vector.tensor_copy(
                    v_sb[:, tt, :, 0:D],
                    ps[:].rearrange("p (h d) -> p h d", h=HL),
                )

            def bg_chain():
                # v is consumed by attn@v from iteration 0 onward
                for tt in range(TT):
                    yield from gen_v(tt)
                # k[p] is needed in full at iteration 4p; q[p,sc] at 4p+sc
                for nt in range(1, NT):
                    for sc in range(SC):
                        yield from gen_qk(wk, bk_sb, kT, nt, sc)
                    for sc in range(SC):
                        yield from gen_qk(wq, bq_sb, qT, nt, sc)

            BG_JIT = os.environ.get("BG_JIT", "1") == "1"
            if BG_JIT:
                bg = bg_chain()
            else:
                bg = iter(())

            def inject(n):
                for _ in range(n):
                    if next(bg, "done") == "done":
                        break

            # ---- preamble: q/k for pair 0 (dense, warms HAM) ----
            for sc in range(SC):
                for _ in gen_qk(wq, bq_sb, qT, 0, sc):
                    pass
                for _ in gen_qk(wk, bk_sb, kT, 0, sc):
                    pass
            if not BG_JIT:
                for _ in bg_chain():
                    pass

            # ---- attention: ScalarE-paced pipeline over (pair, s-chunk) ----
            def emit_scores_group(p, sc, g):
                """4 score MMs (h0/h64 adjacent pairs) + 2 wide exps."""
                ssl = slice(SCW * sc, SCW * (sc + 1))
                sh = [ps_s.tile([128, 2, SCW], f32, tag="s", name=f"s{i}")
                      for i in range(2)]
                if os.environ.get("SCORE_PAIR", "1") == "1":
                    # h0/h64 adjacent -> concurrent row-group execution
                    for j in range(2):
                        tt = 2 * g + j
                        tsl = slice(128 * tt, 128 * (tt + 1))
                        nc.tensor.matmul(sh[0][:, j, :], kT[0:64, p, tsl],
                                         qT[0:64, p, ssl], start=True, stop=True)
                        nc.tensor.matmul(sh[1][:, j, :], kT[64:128, p, tsl],
                                         qT[64:128, p, ssl], start=True, stop=True)
                else:
                    for i in range(2):
                        o = 64 * i
                        for j in range(2):
                            tt = 2 * g + j
                            tsl = slice(128 * tt, 128 * (tt + 1))
                            nc.tensor.matmul(sh[i][:, j, :],
                                             kT[o:o + 64, p, tsl],
                                             qT[o:o + 64, p, ssl],
                                             start=True, stop=True)
                return sh

            def emit_exps(sh):
                ets = [etp.tile([128, 2, SCW], bf16, tag="et", name=f"et{i}")
                       for i in range(2)]
                for i in range(2):
                    nc.scalar.activation(ets[i][:], sh[i][:], AF.Exp,
                                         scale=float(SCALE))
                return ets

            def emit_attnv_group(p, g, ets, un):
                for j in range(2):
                    tt = 2 * g + j
                    for i in range(2):
                        nc.tensor.matmul(
                            un[0:D + 1, i, :],
                            v_sb[:, tt, 2 * p + i, :],
                            ets[i][:, j, :],
                            start=(tt == 0), stop=(tt == TT - 1),
                        )

            def emit_norm(p, sc, un):
                ssl = slice(SCW * sc, SCW * (sc + 1))
                un_sb = small.tile([128, SCW], f32, tag="un_sb")
                nc.vector.tensor_copy(un_sb[0:D, :], un[0:D, 0, :])
                nc.vector.tensor_copy(un_sb[64:64 + D, :], un[0:D, 1, :])
                denom = small.tile([1, 2, SCW], f32, tag="denom")
                nc.vector.tensor_copy(denom[:], un[D:D + 1, :, :])
                rb = small.tile([1, 2, SCW], f32, tag="rb")
                nc.vector.reciprocal_approx_fast(rb[:], denom[:])
                rb16 = small.tile([1, 2, SCW], bf16, tag="rb16")
                nc.vector.tensor_copy(rb16[:], rb[:])
                bc = ps_w.tile([128, SCW], f32, tag="proj")
                nc.tensor.matmul(bc[0:D, :], ones_sb[:], rb16[0:1, 0, :],
                                 start=True, stop=True)
                nc.tensor.matmul(bc[64:64 + D, :], ones_sb[:], rb16[0:1, 1, :],
                                 start=True, stop=True)
                nc.vector.tensor_mul(aT[:, p, ssl], un_sb[:], bc[:])
                nc.vector.tensor_scalar_add(aT[:, p, ssl], aT[:, p, ssl],
                                            bv_sb[:, p:p + 1])

            def emit_gather(half):
                cc_in = dram.tile([128, 2, S], bf16, name=f"cc_in{half}")
                cc_out = dram.tile([2, 128, 2, S], bf16, name=f"cc_out{half}")
                nc.sync.dma_start(cc_in[:], aT[:, 2 * half:2 * half + 2, :])
                nc.gpsimd.collective_compute(
                    "AllGather",
                    mybir.AluOpType.bypass,
                    replica_groups=REPLICA_GROUPS,
                    ins=[cc_in[:].opt()],
                    outs=[cc_out[:].opt()],
                )
                for r in range(2):
                    nc.sync.dma_start(
                        aT_full[:, 4 * r + 2 * half:4 * r + 2 * half + 2, :],
                        cc_out[r],
                    )

            for p in range(NT):
                for sc in range(SC):
                    un = ps_un.tile([128, 2, SCW], f32, tag="un")
                    prev = None
                    for g in range(NG):
                        sh = emit_scores_group(p, sc, g)
                        if prev is not None:
                            emit_attnv_group(p, g - 1, prev, un)
                        ets = emit_exps(sh)
                        inject(4)
                        prev = ets
                    emit_attnv_group(p, NG - 1, prev, un)
                    emit_norm(p, sc, un)
                    if sc == SC - 1 and p in (1, NT - 1):
                        emit_gather(0 if p == 1 else 1)

            # ---- out-projection: o^T[e_tile, s] = Wo^T.T @ aT_full ----
            for et_i in range(NT):
                for sc in range(SC):
                    ssl = slice(SCW * sc, SCW * (sc + 1))
                    po = ps_w.tile([128, SCW], f32, tag="proj")
                    for k, cc in enumerate(CC_ORDER):
                        nc.tensor.matmul(
                            po[:],
                            wo[:, cc, 128 * et_i:128 * (et_i + 1)],
                            aT_full[:, cc, ssl],
                            start=(k == 0), stop=(k == EC - 1),
                        )
                    o_st = small.tile([128, SCW], f32, tag="o_st")
                    nc.vector.tensor_scalar_add(
                        o_st[:], po[:], bo_sb[:, et_i:et_i + 1])
                    nc.sync.dma_start(out_ext[et_i, :, ssl], o_st[:])
    nc.compile()
    return nc


def _prep_inputs(x, Wq, bq, Wk, bk, Wv, bv, Wo, bo):
    """Shard + lay out the full inputs for the 8 cores."""
    import ml_dtypes
    bfl = ml_dtypes.bfloat16

    in_maps = []
    for c in range(N_CORES):
        b, g = divmod(c, G)
        ns = slice(NS * g, NS * (g + 1))
        xT = np.ascontiguousarray(x[b].T).astype(bfl).reshape(EC, 128, S)
        wq_l = np.ascontiguousarray(Wq[ns, :].T).astype(bfl).reshape(EC, 128, NS)
        wk_l = np.ascontiguousarray(Wk[ns, :].T).astype(bfl).reshape(EC, 128, NS)
        wv_l = np.ascontiguousarray(Wv[ns, :].T).astype(bfl).reshape(EC, 128, NS)
        # Wo^T rows (contraction n) in gathered order, cols = this core's
        # e-slice
        woT = np.ascontiguousarray(Wo[ns, :].T)  # [E, NS] = Wo.T[:, es]
        woT = woT.reshape(EC, 128, NS)
        wo_l = woT.astype(bfl).reshape(EC, 128, NS)
        bq_l = np.ascontiguousarray(bq[ns].reshape(NT, 128).T).astype(np.float32)
        bk_l = np.ascontiguousarray(bk[ns].reshape(NT, 128).T).astype(np.float32)
        bv_l = np.ascontiguousarray(bv[ns].reshape(NT, 128).T).astype(np.float32)
        bo_l = np.ascontiguousarray(bo[ns].reshape(NT, 128).T).astype(np.float32)
        in_maps.append({
            "xT": np.ascontiguousarray(xT),
            "wq": np.ascontiguousarray(wq_l),
            "wk": np.ascontiguousarray(wk_l),
            "wv": np.ascontiguousarray(wv_l),
            "wo": np.ascontiguousarray(wo_l),
            "bq": bq_l, "bk": bk_l, "bv": bv_l, "bo": bo_l,
        })
    return in_maps


def kernel(x, Wq, bq, Wk, bk, Wv, bv, Wo, bo, _trace=False):
    x = np.asarray(x)
    in_maps = _prep_inputs(np.asarray(x, np.float32),
                           *[np.asarray(a, np.float32)
                             for a in (Wq, bq, Wk, bk, Wv, bv, Wo, bo)])
    if "nc" not in _CACHE:
        _CACHE["nc"] = build()
    nc = _CACHE["nc"]
    res = run_bass_kernel_spmd(nc, in_maps, core_ids=list(range(N_CORES)),
                               trace=_trace)
    _CACHE["last_result"] = res

    out = np.empty((B, S, E), np.float32)
    for c in range(N_CORES):
        b, g = divmod(c, G)
        oT = res.results[c]["out"].reshape(NS, S)  # [e_sub, s]
        out[b, :, NS * g:NS * (g + 1)] = oT.T
    return out


if __name__ == "__main__":
    nc = build()
    print("built ok:", len(nc.inst_map), "instructions")


# revision 8
# speedup vs baseline: 1.4349x; 1.1102x over previous
"""Distributed multi-head attention block on 8 TRN2 NeuronCores.

Problem: B=4, S=2048, E=1024, H=16 heads, D=64.
Sharding: core c -> (batch b = c//2, head-group g = c%2 of 8 heads).

v2 pipeline: one global software pipeline. Attention is ScalarE(exp)-
bound, so all projection matmuls (full 128x128 array utilization) are
injected into the PE idle slots of the attention loop. This keeps the
PE HAM activity monitor at K=8/8 (2.4 GHz) -- the v1 kernel ran the
whole attention phase at K=4/8 because its 64-row/65-col matmuls with
idle gaps read as ~50% utilization.

Scores h0/h64 are issued adjacently so the two 64-row matmuls execute
concurrently in separate PE row-groups. Exp reads 2 PSUM banks per
ACTIVATE (N=1024) to amortize the 352-cycle ACT overhead. Softmax
denominators ride the ones-column appended to V; 1/denom via the fast
DVE reciprocal on the compact [1,2,512] row, then broadcast via rank-1
PE matmuls.
"""

import os
import sys

sys.path.insert(0, "/opt/trn_rl_repo")

import numpy as np

import concourse.bass as bass
import concourse.bacc as bacc
import concourse.mybir as mybir
import concourse.tile as tile
from concourse.bass_utils import run_bass_kernel_spmd

bf16 = mybir.dt.bfloat16
f32 = mybir.dt.float32
AF = mybir.ActivationFunctionType

N_CORES = 8

# Full problem dims
B, S, E, H, D = 4, 2048, 1024, 16, 64
G = 2            # head-groups (tensor-parallel degree within a batch)
NS = E // G      # 512: n-dims (head dims) per core
HL = H // G      # 8 heads per core
EC = E // 128    # 8 contraction chunks for projections
NT = NS // 128   # 4 tiles of q^T/k^T (= head pairs)
TT = S // 128    # 16 t-tiles
SCW = 512        # s-chunk width
SC = S // SCW    # 4 s-chunks
NG = TT // 2     # 8 tt-groups of 2 per attention iteration
SCALE = 1.0 / np.sqrt(D)

REPLICA_GROUPS = [[2 * i, 2 * i + 1] for i in range(4)]

# aT_full chunk order (gather writes row r of half h to slot 4r+2h)
CC_ORDER = [0, 1, 4, 5, 2, 3, 6, 7]  # gather-0 chunks first

_CACHE = {}


def build(debug=False):
    """Build the SPMD bass graph (identical on all 8 cores)."""
    nc = bacc.Bacc("TRN2", target_bir_lowering=False, debug=debug,
                   num_devices=N_CORES)

    # --- per-core external I/O (shards prepared host-side) ---
    x_ext = nc.dram_tensor("xT", [EC, 128, S], bf16, kind="ExternalInput")
    wq_ext = nc.dram_tensor("wq", [EC, 128, NS], bf16, kind="ExternalInput")
    wk_ext = nc.dram_tensor("wk", [EC, 128, NS], bf16, kind="ExternalInput")
    wv_ext = nc.dram_tensor("wv", [EC, 128, NS], bf16, kind="ExternalInput")
    wo_ext = nc.dram_tensor("wo", [EC, 128, NS], bf16, kind="ExternalInput")
    bq_ext = nc.dram_tensor("bq", [128, NT], f32, kind="ExternalInput")
    bk_ext = nc.dram_tensor("bk", [128, NT], f32, kind="ExternalInput")
    bv_ext = nc.dram_tensor("bv", [128, NT], f32, kind="ExternalInput")
    bo_ext = nc.dram_tensor("bo", [128, NT], f32, kind="ExternalInput")
    out_ext = nc.dram_tensor("out", [NT, 128, S], f32, kind="ExternalOutput")

    with tile.TileContext(nc) as tc:
        with (
            tc.tile_pool(name="big", bufs=1) as big,        # persistent sbuf
            tc.tile_pool(name="et", bufs=8) as etp,         # E^T tiles
            tc.tile_pool(name="small", bufs=2) as small,    # rows / tmp
            tc.tile_pool(name="ps_s", bufs=2, space="PSUM") as ps_s,    # 4 banks
            tc.tile_pool(name="ps_w", bufs=2, space="PSUM") as ps_w,    # 2 banks
            tc.tile_pool(name="ps_un", bufs=1, space="PSUM") as ps_un,  # 2 banks
            tc.tile_pool(name="dram", bufs=1, space="DRAM") as dram,
        ):
            # ---- persistent SBUF tensors ----
            xT = big.tile([128, EC, S], bf16)                  # x^T  (e, s)
            wq = big.tile([128, EC, NS], bf16)
            wk = big.tile([128, EC, NS], bf16)
            wv = big.tile([128, EC, NS], bf16)
            wo = big.tile([128, EC, NS], bf16)
            bq_sb = big.tile([128, NT], f32)
            bk_sb = big.tile([128, NT], f32)
            bv_sb = big.tile([128, NT], f32)
            bo_sb = big.tile([128, NT], f32)
            qT = big.tile([128, NT, S], bf16)                  # q^T  (n, s)
            kT = big.tile([128, NT, S], bf16)                  # k^T  (n, t)
            v_sb = big.tile([128, TT, HL, D + 1], bf16)        # v (+ones col)
            aT = big.tile([128, NT, S], bf16)                  # attn out^T (n, s)
            aT_full = big.tile([128, EC, S], bf16)             # gathered attn^T
            ones_sb = big.tile([1, D], bf16)

            for c in range(EC):
                nc.sync.dma_start(xT[:, c, :], x_ext[c])
            nc.sync.dma_start(wq[:], wq_ext[:].rearrange("c p n -> p c n"))
            nc.sync.dma_start(wk[:], wk_ext[:].rearrange("c p n -> p c n"))
            nc.sync.dma_start(wv[:], wv_ext[:].rearrange("c p n -> p c n"))
            nc.sync.dma_start(wo[:], wo_ext[:].rearrange("c p n -> p c n"))
            nc.sync.dma_start(bq_sb[:], bq_ext[:])
            nc.sync.dma_start(bk_sb[:], bk_ext[:])
            nc.sync.dma_start(bv_sb[:], bv_ext[:])
            nc.sync.dma_start(bo_sb[:], bo_ext[:])
            nc.vector.memset(ones_sb[:], 1.0)
            nc.vector.memset(v_sb[:, :, :, D:D + 1], 1.0)

            # ---- background projection generators (one yield per MM) ----
            def gen_qk(w_sb, b_sb, dst, nt, sc):
                ssl = slice(SCW * sc, SCW * (sc + 1))
                ps = ps_w.tile([128, SCW], f32, tag="proj")
                for ec in range(EC):
                    nc.tensor.matmul(
                        ps[:],
                        w_sb[:, ec, 128 * nt:128 * (nt + 1)],
                        xT[:, ec, ssl],
                        start=(ec == 0), stop=(ec == EC - 1),
                    )
                    yield
                nc.vector.tensor_scalar_add(dst[:, nt, ssl], ps[:],
                                            b_sb[:, nt:nt + 1])

            def gen_v(tt):
                ps = ps_w.tile([128, NS], f32, tag="proj")
                for ec in range(EC):
                    nc.tensor.matmul(
                        ps[:],
                        xT[:, ec, 128 * tt:128 * (tt + 1)],
                        wv[:, ec, :],
                        start=(ec == 0), stop=(ec == EC - 1),
                    )
                    yield
                nc.vector.tensor_copy(
                    v_sb[:, tt, :, 0:D],
                    ps[:].rearrange("p (h d) -> p h d", h=HL),
                )

            def bg_chain():
                # kT is a *stationary* operand: emit all of it first so the
                # producing evict always leads the consuming LDWEIGHTS by
                # well over the PE's 64-deep reorder window. qT is a moving
                # operand (streamed at MM execution) -- safe just-in-time.
                for nt in range(1, NT):
                    for sc in range(SC):
                        yield from gen_qk(wk, bk_sb, kT, nt, sc)
                for nt in range(1, NT):
                    for sc in range(SC):
                        yield from gen_qk(wq, bq_sb, qT, nt, sc)

            BG_JIT = os.environ.get("BG_JIT", "1") == "1"
            if BG_JIT:
                bg = bg_chain()
            else:
                bg = iter(())

            def inject(n):
                for _ in range(n):
                    if next(bg, "done") == "done":
                        break

            # ---- preamble: q/k pair 0 + all of v (dense, warms HAM).
            # v is a stationary operand consumed from iteration 0 -- it
            # cannot get a safe lead in the background stream.
            for sc in range(SC):
                for _ in gen_qk(wq, bq_sb, qT, 0, sc):
                    pass
                for _ in gen_qk(wk, bk_sb, kT, 0, sc):
                    pass
            for tt in range(TT):
                for _ in gen_v(tt):
                    pass
            if not BG_JIT:
                for _ in bg_chain():
                    pass

            # ---- attention: ScalarE-paced pipeline over (pair, s-chunk) ----
            def emit_scores_group(p, sc, g):
                """4 score MMs (h0/h64 adjacent pairs) + 2 wide exps."""
                ssl = slice(SCW * sc, SCW * (sc + 1))
                sh = [ps_s.tile([128, 2, SCW], f32, tag="s", name=f"s{i}")
                      for i in range(2)]
                if os.environ.get("SCORE_PAIR", "1") == "1":
                    # h0/h64 adjacent -> concurrent row-group execution
                    for j in range(2):
                        tt = 2 * g + j
                        tsl = slice(128 * tt, 128 * (tt + 1))
                        nc.tensor.matmul(sh[0][:, j, :], kT[0:64, p, tsl],
                                         qT[0:64, p, ssl], start=True, stop=True)
                        nc.tensor.matmul(sh[1][:, j, :], kT[64:128, p, tsl],
                                         qT[64:128, p, ssl], start=True, stop=True)
                else:
                    for i in range(2):
                        o = 64 * i
                        for j in range(2):
                            tt = 2 * g + j
                            tsl = slice(128 * tt, 128 * (tt + 1))
                            nc.tensor.matmul(sh[i][:, j, :],
                                             kT[o:o + 64, p, tsl],
                                             qT[o:o + 64, p, ssl],
                                             start=True, stop=True)
                return sh

            def emit_exps(sh):
                ets = [etp.tile([128, 2, SCW], bf16, tag="et", name=f"et{i}")
                       for i in range(2)]
                for i in range(2):
                    nc.scalar.activation(ets[i][:], sh[i][:], AF.Exp,
                                         scale=float(SCALE))
                return ets

            def emit_attnv_group(p, g, ets, un):
                for j in range(2):
                    tt = 2 * g + j
                    for i in range(2):
                        nc.tensor.matmul(
                            un[0:D + 1, i, :],
                            v_sb[:, tt, 2 * p + i, :],
                            ets[i][:, j, :],
                            start=(tt == 0), stop=(tt == TT - 1),
                        )

            def emit_norm(p, sc, un):
                ssl = slice(SCW * sc, SCW * (sc + 1))
                un_sb = small.tile([128, SCW], f32, tag="un_sb")
                nc.vector.tensor_copy(un_sb[0:D, :], un[0:D, 0, :])
                nc.vector.tensor_copy(un_sb[64:64 + D, :], un[0:D, 1, :])
                denom = small.tile([1, 2, SCW], f32, tag="denom")
                nc.vector.tensor_copy(denom[:], un[D:D + 1, :, :])
                rb = small.tile([1, 2, SCW], f32, tag="rb")
                nc.vector.reciprocal_approx_fast(rb[:], denom[:])
                rb16 = small.tile([1, 2, SCW], bf16, tag="rb16")
                nc.vector.tensor_copy(rb16[:], rb[:])
                bc = ps_w.tile([128, SCW], f32, tag="proj")
                nc.tensor.matmul(bc[0:D, :], ones_sb[:], rb16[0:1, 0, :],
                                 start=True, stop=True)
                nc.tensor.matmul(bc[64:64 + D, :], ones_sb[:], rb16[0:1, 1, :],
                                 start=True, stop=True)
                nc.vector.tensor_mul(aT[:, p, ssl], un_sb[:], bc[:])
                nc.vector.tensor_scalar_add(aT[:, p, ssl], aT[:, p, ssl],
                                            bv_sb[:, p:p + 1])

            def emit_gather(half):
                cc_in = dram.tile([128, 2, S], bf16, name=f"cc_in{half}")
                cc_out = dram.tile([2, 128, 2, S], bf16, name=f"cc_out{half}")
                nc.sync.dma_start(cc_in[:], aT[:, 2 * half:2 * half + 2, :])
                nc.gpsimd.collective_compute(
                    "AllGather",
                    mybir.AluOpType.bypass,
                    replica_groups=REPLICA_GROUPS,
                    ins=[cc_in[:].opt()],
                    outs=[cc_out[:].opt()],
                )
                for r in range(2):
                    nc.sync.dma_start(
                        aT_full[:, 4 * r + 2 * half:4 * r + 2 * half + 2, :],
                        cc_out[r],
                    )

            for p in range(NT):
                for sc in range(SC):
                    un = ps_un.tile([128, 2, SCW], f32, tag="un")
                    prev = None
                    for g in range(NG):
                        sh = emit_scores_group(p, sc, g)
                        if prev is not None:
                            emit_attnv_group(p, g - 1, prev, un)
                        ets = emit_exps(sh)
                        inject(4)
                        prev = ets
                    emit_attnv_group(p, NG - 1, prev, un)
                    emit_norm(p, sc, un)
                    if sc == SC - 1 and p in (1, NT - 1):
                        emit_gather(0 if p == 1 else 1)

            # ---- out-projection: o^T[e_tile, s] = Wo^T.T @ aT_full ----
            for et_i in range(NT):
                for sc in range(SC):
                    ssl = slice(SCW * sc, SCW * (sc + 1))
                    po = ps_w.tile([128, SCW], f32, tag="proj")
                    for k, cc in enumerate(CC_ORDER):
                        nc.tensor.matmul(
                            po[:],
                            wo[:, cc, 128 * et_i:128 * (et_i + 1)],
                            aT_full[:, cc, ssl],
                            start=(k == 0), stop=(k == EC - 1),
                        )
                    o_st = small.tile([128, SCW], f32, tag="o_st")
                    nc.vector.tensor_scalar_add(
                        o_st[:], po[:], bo_sb[:, et_i:et_i + 1])
                    nc.sync.dma_start(out_ext[et_i, :, ssl], o_st[:])
    nc.compile()
    return nc


def _prep_inputs(x, Wq, bq, Wk, bk, Wv, bv, Wo, bo):
    """Shard + lay out the full inputs for the 8 cores."""
    import ml_dtypes
    bfl = ml_dtypes.bfloat16

    in_maps = []
    for c in range(N_CORES):
        b, g = divmod(c, G)
        ns = slice(NS * g, NS * (g + 1))
        xT = np.ascontiguousarray(x[b].T).astype(bfl).reshape(EC, 128, S)
        wq_l = np.ascontiguousarray(Wq[ns, :].T).astype(bfl).reshape(EC, 128, NS)
        wk_l = np.ascontiguousarray(Wk[ns, :].T).astype(bfl).reshape(EC, 128, NS)
        wv_l = np.ascontiguousarray(Wv[ns, :].T).astype(bfl).reshape(EC, 128, NS)
        # Wo^T rows (contraction n) in gathered order, cols = this core's
        # e-slice
        woT = np.ascontiguousarray(Wo[ns, :].T)  # [E, NS] = Wo.T[:, es]
        woT = woT.reshape(EC, 128, NS)
        wo_l = woT.astype(bfl).reshape(EC, 128, NS)
        bq_l = np.ascontiguousarray(bq[ns].reshape(NT, 128).T).astype(np.float32)
        bk_l = np.ascontiguousarray(bk[ns].reshape(NT, 128).T).astype(np.float32)
        bv_l = np.ascontiguousarray(bv[ns].reshape(NT, 128).T).astype(np.float32)
        bo_l = np.ascontiguousarray(bo[ns].reshape(NT, 128).T).astype(np.float32)
        in_maps.append({
            "xT": np.ascontiguousarray(xT),
            "wq": np.ascontiguousarray(wq_l),
            "wk": np.ascontiguousarray(wk_l),
            "wv": np.ascontiguousarray(wv_l),
            "wo": np.ascontiguousarray(wo_l),
            "bq": bq_l, "bk": bk_l, "bv": bv_l, "bo": bo_l,
        })
    return in_maps


def kernel(x, Wq, bq, Wk, bk, Wv, bv, Wo, bo, _trace=False):
    x = np.asarray(x)
    in_maps = _prep_inputs(np.asarray(x, np.float32),
                           *[np.asarray(a, np.float32)
                             for a in (Wq, bq, Wk, bk, Wv, bv, Wo, bo)])
    if "nc" not in _CACHE:
        _CACHE["nc"] = build()
    nc = _CACHE["nc"]
    res = run_bass_kernel_spmd(nc, in_maps, core_ids=list(range(N_CORES)),
                               trace=_trace)
    _CACHE["last_result"] = res

    out = np.empty((B, S, E), np.float32)
    for c in range(N_CORES):
        b, g = divmod(c, G)
        oT = res.results[c]["out"].reshape(NS, S)  # [e_sub, s]
        out[b, :, NS * g:NS * (g + 1)] = oT.T
    return out


if __name__ == "__main__":
    nc = build()
    print("built ok:", len(nc.inst_map), "instructions")


# revision 9
# speedup vs baseline: 1.5013x; 1.0462x over previous
"""Distributed multi-head attention block on 8 TRN2 NeuronCores.

Problem: B=4, S=2048, E=1024, H=16 heads, D=64.
Sharding: core c -> (batch b = c//2, head-group g = c%2 of 8 heads).

v2 pipeline: one global software pipeline. Attention is ScalarE(exp)-
bound, so all projection matmuls (full 128x128 array utilization) are
injected into the PE idle slots of the attention loop. This keeps the
PE HAM activity monitor at K=8/8 (2.4 GHz) -- the v1 kernel ran the
whole attention phase at K=4/8 because its 64-row/65-col matmuls with
idle gaps read as ~50% utilization.

Scores h0/h64 are issued adjacently so the two 64-row matmuls execute
concurrently in separate PE row-groups. Exp reads 2 PSUM banks per
ACTIVATE (N=1024) to amortize the 352-cycle ACT overhead. Softmax
denominators ride the ones-column appended to V; 1/denom via the fast
DVE reciprocal on the compact [1,2,512] row, then broadcast via rank-1
PE matmuls.
"""

import os
import sys

sys.path.insert(0, "/opt/trn_rl_repo")

import numpy as np

import concourse.bass as bass
import concourse.bacc as bacc
import concourse.mybir as mybir
import concourse.tile as tile
from concourse.bass_utils import run_bass_kernel_spmd

bf16 = mybir.dt.bfloat16
f32 = mybir.dt.float32
AF = mybir.ActivationFunctionType

N_CORES = 8

# Full problem dims
B, S, E, H, D = 4, 2048, 1024, 16, 64
G = 2            # head-groups (tensor-parallel degree within a batch)
NS = E // G      # 512: n-dims (head dims) per core
HL = H // G      # 8 heads per core
EC = E // 128    # 8 contraction chunks for projections
NT = NS // 128   # 4 tiles of q^T/k^T (= head pairs)
TT = S // 128    # 16 t-tiles
SCW = 512        # s-chunk width
SC = S // SCW    # 4 s-chunks
NG = TT // 2     # 8 tt-groups of 2 per attention iteration
SCALE = 1.0 / np.sqrt(D)

REPLICA_GROUPS = [[2 * i, 2 * i + 1] for i in range(4)]

# Per-pair gathers: pair p's AllGather writes replica-row r into aT_full
# slot 2p+r, i.e. slot order = [c0p0, c1p0, c0p1, c1p1, ...]. Host-side
# wo rows are permuted to match (global chunk 4r+p at slot 2p+r).
WO_ORDER = [0, 4, 1, 5, 2, 6, 3, 7]

_CACHE = {}


def build(debug=False):
    """Build the SPMD bass graph (identical on all 8 cores)."""
    nc = bacc.Bacc("TRN2", target_bir_lowering=False, debug=debug,
                   num_devices=N_CORES)

    # --- per-core external I/O (shards prepared host-side) ---
    x_ext = nc.dram_tensor("xT", [EC, 128, S], bf16, kind="ExternalInput")
    wq_ext = nc.dram_tensor("wq", [EC, 128, NS], bf16, kind="ExternalInput")
    wk_ext = nc.dram_tensor("wk", [EC, 128, NS], bf16, kind="ExternalInput")
    wv_ext = nc.dram_tensor("wv", [EC, 128, NS], bf16, kind="ExternalInput")
    wo_ext = nc.dram_tensor("wo", [EC, 128, NS], bf16, kind="ExternalInput")
    bq_ext = nc.dram_tensor("bq", [128, NT], f32, kind="ExternalInput")
    bk_ext = nc.dram_tensor("bk", [128, NT], f32, kind="ExternalInput")
    bv_ext = nc.dram_tensor("bv", [128, NT], f32, kind="ExternalInput")
    bo_ext = nc.dram_tensor("bo", [128, NT], f32, kind="ExternalInput")
    out_ext = nc.dram_tensor("out", [NT, 128, S], f32, kind="ExternalOutput")

    with tile.TileContext(nc) as tc:
        with (
            tc.tile_pool(name="big", bufs=1) as big,        # persistent sbuf
            tc.tile_pool(name="et", bufs=8) as etp,         # E^T tiles
            tc.tile_pool(name="small", bufs=2) as small,    # rows / tmp
            tc.tile_pool(name="ps_s", bufs=2, space="PSUM") as ps_s,    # 4 banks
            tc.tile_pool(name="ps_w", bufs=2, space="PSUM") as ps_w,    # 2 banks
            tc.tile_pool(name="ps_un", bufs=1, space="PSUM") as ps_un,  # 2 banks
            tc.tile_pool(name="dram", bufs=1, space="DRAM") as dram,
        ):
            # ---- persistent SBUF tensors ----
            xT = big.tile([128, EC, S], bf16)                  # x^T  (e, s)
            wq = big.tile([128, EC, NS], bf16)
            wk = big.tile([128, EC, NS], bf16)
            wv = big.tile([128, EC, NS], bf16)
            wo = big.tile([128, EC, NS], bf16)
            bq_sb = big.tile([128, NT], f32)
            bk_sb = big.tile([128, NT], f32)
            bv_sb = big.tile([128, NT], f32)
            bo_sb = big.tile([128, NT], f32)
            qT = big.tile([128, NT, S], bf16)                  # q^T  (n, s)
            kT = big.tile([128, NT, S], bf16)                  # k^T  (n, t)
            v_sb = big.tile([128, TT, HL, D + 1], bf16)        # v (+ones col)
            aT = big.tile([128, NT, S], bf16)                  # attn out^T (n, s)
            aT_full = big.tile([128, EC, S], bf16)             # gathered attn^T
            ones_sb = big.tile([1, D], bf16)

            for c in range(EC):
                nc.sync.dma_start(xT[:, c, :], x_ext[c])
            nc.sync.dma_start(wq[:], wq_ext[:].rearrange("c p n -> p c n"))
            nc.sync.dma_start(wk[:], wk_ext[:].rearrange("c p n -> p c n"))
            nc.sync.dma_start(wv[:], wv_ext[:].rearrange("c p n -> p c n"))
            nc.sync.dma_start(wo[:], wo_ext[:].rearrange("c p n -> p c n"))
            nc.sync.dma_start(bq_sb[:], bq_ext[:])
            nc.sync.dma_start(bk_sb[:], bk_ext[:])
            nc.sync.dma_start(bv_sb[:], bv_ext[:])
            nc.sync.dma_start(bo_sb[:], bo_ext[:])
            nc.vector.memset(ones_sb[:], 1.0)
            nc.vector.memset(v_sb[:, :, :, D:D + 1], 1.0)

            # ---- background projection generators (one yield per MM) ----
            def gen_qk(w_sb, b_sb, dst, nt, sc):
                ssl = slice(SCW * sc, SCW * (sc + 1))
                ps = ps_w.tile([128, SCW], f32, tag="proj")
                for ec in range(EC):
                    nc.tensor.matmul(
                        ps[:],
                        w_sb[:, ec, 128 * nt:128 * (nt + 1)],
                        xT[:, ec, ssl],
                        start=(ec == 0), stop=(ec == EC - 1),
                    )
                    yield
                nc.vector.tensor_scalar_add(dst[:, nt, ssl], ps[:],
                                            b_sb[:, nt:nt + 1])

            def gen_v(tt):
                ps = ps_w.tile([128, NS], f32, tag="proj")
                for ec in range(EC):
                    nc.tensor.matmul(
                        ps[:],
                        xT[:, ec, 128 * tt:128 * (tt + 1)],
                        wv[:, ec, :],
                        start=(ec == 0), stop=(ec == EC - 1),
                    )
                    yield
                nc.vector.tensor_copy(
                    v_sb[:, tt, :, 0:D],
                    ps[:].rearrange("p (h d) -> p h d", h=HL),
                )

            def bg_chain():
                # kT is a *stationary* operand: emit all of it first so the
                # producing evict always leads the consuming LDWEIGHTS by
                # well over the PE's 64-deep reorder window. qT is a moving
                # operand (streamed at MM execution) -- safe just-in-time.
                for nt in range(1, NT):
                    for sc in range(SC):
                        yield from gen_qk(wk, bk_sb, kT, nt, sc)
                for nt in range(1, NT):
                    for sc in range(SC):
                        yield from gen_qk(wq, bq_sb, qT, nt, sc)

            BG_JIT = os.environ.get("BG_JIT", "1") == "1"
            if BG_JIT:
                bg = bg_chain()
            else:
                bg = iter(())

            def inject(n):
                for _ in range(n):
                    if next(bg, "done") == "done":
                        break

            # ---- preamble: q/k pair 0 + all of v (dense, warms HAM).
            # v is a stationary operand consumed from iteration 0 -- it
            # cannot get a safe lead in the background stream.
            for sc in range(SC):
                for _ in gen_qk(wq, bq_sb, qT, 0, sc):
                    pass
                for _ in gen_qk(wk, bk_sb, kT, 0, sc):
                    pass
            for tt in range(TT):
                for _ in gen_v(tt):
                    pass
            if not BG_JIT:
                for _ in bg_chain():
                    pass

            # ---- attention: ScalarE-paced pipeline over (pair, s-chunk) ----
            def emit_scores_group(p, sc, g):
                """4 score MMs (h0/h64 adjacent pairs) + 2 wide exps."""
                ssl = slice(SCW * sc, SCW * (sc + 1))
                sh = [ps_s.tile([128, 2, SCW], f32, tag="s", name=f"s{i}")
                      for i in range(2)]
                if os.environ.get("SCORE_PAIR", "1") == "1":
                    # h0/h64 adjacent -> concurrent row-group execution
                    for j in range(2):
                        tt = 2 * g + j
                        tsl = slice(128 * tt, 128 * (tt + 1))
                        nc.tensor.matmul(sh[0][:, j, :], kT[0:64, p, tsl],
                                         qT[0:64, p, ssl], start=True, stop=True)
                        nc.tensor.matmul(sh[1][:, j, :], kT[64:128, p, tsl],
                                         qT[64:128, p, ssl], start=True, stop=True)
                else:
                    for i in range(2):
                        o = 64 * i
                        for j in range(2):
                            tt = 2 * g + j
                            tsl = slice(128 * tt, 128 * (tt + 1))
                            nc.tensor.matmul(sh[i][:, j, :],
                                             kT[o:o + 64, p, tsl],
                                             qT[o:o + 64, p, ssl],
                                             start=True, stop=True)
                return sh

            def emit_exps(sh):
                ets = [etp.tile([128, 2, SCW], bf16, tag="et", name=f"et{i}")
                       for i in range(2)]
                for i in range(2):
                    nc.scalar.activation(ets[i][:], sh[i][:], AF.Exp,
                                         scale=float(SCALE))
                return ets

            def emit_attnv_group(p, g, ets, un):
                for j in range(2):
                    tt = 2 * g + j
                    for i in range(2):
                        nc.tensor.matmul(
                            un[0:D + 1, i, :],
                            v_sb[:, tt, 2 * p + i, :],
                            ets[i][:, j, :],
                            start=(tt == 0), stop=(tt == TT - 1),
                        )

            def emit_norm(p, sc, un):
                ssl = slice(SCW * sc, SCW * (sc + 1))
                un_sb = small.tile([128, SCW], f32, tag="un_sb")
                nc.vector.tensor_copy(un_sb[0:D, :], un[0:D, 0, :])
                nc.vector.tensor_copy(un_sb[64:64 + D, :], un[0:D, 1, :])
                denom = small.tile([1, 2, SCW], f32, tag="denom")
                nc.vector.tensor_copy(denom[:], un[D:D + 1, :, :])
                rb = small.tile([1, 2, SCW], f32, tag="rb")
                nc.vector.reciprocal_approx_fast(rb[:], denom[:])
                rb16 = small.tile([1, 2, SCW], bf16, tag="rb16")
                nc.vector.tensor_copy(rb16[:], rb[:])
                bc = ps_w.tile([128, SCW], f32, tag="proj")
                nc.tensor.matmul(bc[0:D, :], ones_sb[:], rb16[0:1, 0, :],
                                 start=True, stop=True)
                nc.tensor.matmul(bc[64:64 + D, :], ones_sb[:], rb16[0:1, 1, :],
                                 start=True, stop=True)
                nc.vector.tensor_mul(aT[:, p, ssl], un_sb[:], bc[:])
                nc.vector.tensor_scalar_add(aT[:, p, ssl], aT[:, p, ssl],
                                            bv_sb[:, p:p + 1])

            def emit_gather(half):
                cc_in = dram.tile([128, 2, S], bf16, name=f"cc_in{half}")
                cc_out = dram.tile([2, 128, 2, S], bf16, name=f"cc_out{half}")
                nc.sync.dma_start(cc_in[:], aT[:, 2 * half:2 * half + 2, :])
                nc.gpsimd.collective_compute(
                    "AllGather",
                    mybir.AluOpType.bypass,
                    replica_groups=REPLICA_GROUPS,
                    ins=[cc_in[:].opt()],
                    outs=[cc_out[:].opt()],
                )
                for r in range(2):
                    nc.sync.dma_start(
                        aT_full[:, 4 * r + 2 * half:4 * r + 2 * half + 2, :],
                        cc_out[r],
                    )

            for p in range(NT):
                for sc in range(SC):
                    un = ps_un.tile([128, 2, SCW], f32, tag="un")
                    prev = None
                    for g in range(NG):
                        sh = emit_scores_group(p, sc, g)
                        if prev is not None:
                            emit_attnv_group(p, g - 1, prev, un)
                        ets = emit_exps(sh)
                        inject(4)
                        prev = ets
                    emit_attnv_group(p, NG - 1, prev, un)
                    emit_norm(p, sc, un)
                    if sc == SC - 1 and p in (1, NT - 1):
                        emit_gather(0 if p == 1 else 1)

            # ---- out-projection: o^T[e_tile, s] = Wo^T.T @ aT_full ----
            for et_i in range(NT):
                for sc in range(SC):
                    ssl = slice(SCW * sc, SCW * (sc + 1))
                    po = ps_w.tile([128, SCW], f32, tag="proj")
                    for k, cc in enumerate(CC_ORDER):
                        nc.tensor.matmul(
                            po[:],
                            wo[:, cc, 128 * et_i:128 * (et_i + 1)],
                            aT_full[:, cc, ssl],
                            start=(k == 0), stop=(k == EC - 1),
                        )
                    o_st = small.tile([128, SCW], f32, tag="o_st")
                    nc.vector.tensor_scalar_add(
                        o_st[:], po[:], bo_sb[:, et_i:et_i + 1])
                    nc.sync.dma_start(out_ext[et_i, :, ssl], o_st[:])
    nc.compile()
    return nc


def _prep_inputs(x, Wq, bq, Wk, bk, Wv, bv, Wo, bo):
    """Shard + lay out the full inputs for the 8 cores."""
    import ml_dtypes
    bfl = ml_dtypes.bfloat16

    in_maps = []
    for c in range(N_CORES):
        b, g = divmod(c, G)
        ns = slice(NS * g, NS * (g + 1))
        xT = np.ascontiguousarray(x[b].T).astype(bfl).reshape(EC, 128, S)
        wq_l = np.ascontiguousarray(Wq[ns, :].T).astype(bfl).reshape(EC, 128, NS)
        wk_l = np.ascontiguousarray(Wk[ns, :].T).astype(bfl).reshape(EC, 128, NS)
        wv_l = np.ascontiguousarray(Wv[ns, :].T).astype(bfl).reshape(EC, 128, NS)
        # Wo^T rows (contraction n) in gathered order, cols = this core's
        # e-slice
        woT = np.ascontiguousarray(Wo[ns, :].T)  # [E, NS] = Wo.T[:, es]
        woT = woT.reshape(EC, 128, NS)
        wo_l = woT.astype(bfl).reshape(EC, 128, NS)
        bq_l = np.ascontiguousarray(bq[ns].reshape(NT, 128).T).astype(np.float32)
        bk_l = np.ascontiguousarray(bk[ns].reshape(NT, 128).T).astype(np.float32)
        bv_l = np.ascontiguousarray(bv[ns].reshape(NT, 128).T).astype(np.float32)
        bo_l = np.ascontiguousarray(bo[ns].reshape(NT, 128).T).astype(np.float32)
        in_maps.append({
            "xT": np.ascontiguousarray(xT),
            "wq": np.ascontiguousarray(wq_l),
            "wk": np.ascontiguousarray(wk_l),
            "wv": np.ascontiguousarray(wv_l),
            "wo": np.ascontiguousarray(wo_l),
            "bq": bq_l, "bk": bk_l, "bv": bv_l, "bo": bo_l,
        })
    return in_maps


def kernel(x, Wq, bq, Wk, bk, Wv, bv, Wo, bo, _trace=False):
    x = np.asarray(x)
    in_maps = _prep_inputs(np.asarray(x, np.float32),
                           *[np.asarray(a, np.float32)
                             for a in (Wq, bq, Wk, bk, Wv, bv, Wo, bo)])
    if "nc" not in _CACHE:
        _CACHE["nc"] = build()
    nc = _CACHE["nc"]
    res = run_bass_kernel_spmd(nc, in_maps, core_ids=list(range(N_CORES)),
                               trace=_trace)
    _CACHE["last_result"] = res

    out = np.empty((B, S, E), np.float32)
    for c in range(N_CORES):
        b, g = divmod(c, G)
        oT = res.results[c]["out"].reshape(NS, S)  # [e_sub, s]
        out[b, :, NS * g:NS * (g + 1)] = oT.T
    return out


if __name__ == "__main__":
    nc = build()
    print("built ok:", len(nc.inst_map), "instructions")


# revision 16
# speedup vs baseline: 1.5109x; 1.0064x over previous
"""Distributed multi-head attention block on 8 TRN2 NeuronCores.

Problem: B=4, S=2048, E=1024, H=16 heads, D=64.
Sharding: core c -> (batch b = c//2, head-group g = c%2 of 8 heads).

v2 pipeline: one global software pipeline. Attention is ScalarE(exp)-
bound, so all projection matmuls (full 128x128 array utilization) are
injected into the PE idle slots of the attention loop. This keeps the
PE HAM activity monitor at K=8/8 (2.4 GHz) -- the v1 kernel ran the
whole attention phase at K=4/8 because its 64-row/65-col matmuls with
idle gaps read as ~50% utilization.

Scores h0/h64 are issued adjacently so the two 64-row matmuls execute
concurrently in separate PE row-groups. Exp reads 2 PSUM banks per
ACTIVATE (N=1024) to amortize the 352-cycle ACT overhead. Softmax
denominators ride the ones-column appended to V; 1/denom via the fast
DVE reciprocal on the compact [1,2,512] row, then broadcast via rank-1
PE matmuls.
"""

import os
import sys

sys.path.insert(0, "/opt/trn_rl_repo")

import numpy as np

import concourse.bass as bass
import concourse.bacc as bacc
import concourse.mybir as mybir
import concourse.tile as tile
from concourse.bass_utils import run_bass_kernel_spmd

bf16 = mybir.dt.bfloat16
f32 = mybir.dt.float32
AF = mybir.ActivationFunctionType

N_CORES = 8

# Full problem dims
B, S, E, H, D = 4, 2048, 1024, 16, 64
G = 2            # head-groups (tensor-parallel degree within a batch)
NS = E // G      # 512: n-dims (head dims) per core
HL = H // G      # 8 heads per core
EC = E // 128    # 8 contraction chunks for projections
NT = NS // 128   # 4 tiles of q^T/k^T (= head pairs)
TT = S // 128    # 16 t-tiles
SCW = 512        # s-chunk width
SC = S // SCW    # 4 s-chunks
NG = TT // 2     # 8 tt-groups of 2 per attention iteration
SCALE = 1.0 / np.sqrt(D)

REPLICA_GROUPS = [[2 * i, 2 * i + 1] for i in range(4)]

# Per-pair gathers: pair p's AllGather writes replica-row r into aT_full
# slot 2p+r, i.e. slot order = [c0p0, c1p0, c0p1, c1p1, ...]. Host-side
# wo rows are permuted to match (global chunk 4r+p at slot 2p+r).
WO_ORDER = [0, 4, 1, 5, 2, 6, 3, 7]

_CACHE = {}


def build(debug=False):
    """Build the SPMD bass graph (identical on all 8 cores)."""
    nc = bacc.Bacc("TRN2", target_bir_lowering=False, debug=debug,
                   num_devices=N_CORES)

    # --- per-core external I/O (shards prepared host-side) ---
    x_ext = nc.dram_tensor("xT", [EC, 128, S], bf16, kind="ExternalInput")
    wq_ext = nc.dram_tensor("wq", [EC, 128, NS], bf16, kind="ExternalInput")
    wk_ext = nc.dram_tensor("wk", [EC, 128, NS], bf16, kind="ExternalInput")
    wv_ext = nc.dram_tensor("wv", [EC, 128, NS], bf16, kind="ExternalInput")
    wo_ext = nc.dram_tensor("wo", [EC, 128, NS], bf16, kind="ExternalInput")
    bq_ext = nc.dram_tensor("bq", [128, NT], f32, kind="ExternalInput")
    bk_ext = nc.dram_tensor("bk", [128, NT], f32, kind="ExternalInput")
    bv_ext = nc.dram_tensor("bv", [128, NT], f32, kind="ExternalInput")
    bo_ext = nc.dram_tensor("bo", [128, NT], f32, kind="ExternalInput")
    out_ext = nc.dram_tensor("out", [NT, 128, S], f32, kind="ExternalOutput")

    with tile.TileContext(nc) as tc:
        with (
            tc.tile_pool(name="big", bufs=1) as big,        # persistent sbuf
            tc.tile_pool(name="et", bufs=7) as etp,         # E^T tiles
            tc.tile_pool(name="small", bufs=2) as small,    # rows / tmp
            tc.tile_pool(name="ps_s", bufs=2, space="PSUM") as ps_s,    # 4 banks
            tc.tile_pool(name="ps_w", bufs=2, space="PSUM") as ps_w,    # 2 banks
            tc.tile_pool(name="ps_un", bufs=1, space="PSUM") as ps_un,  # 2 banks
            tc.tile_pool(name="dram", bufs=1, space="DRAM") as dram,
        ):
            # ---- persistent SBUF tensors ----
            xT = big.tile([128, EC, S], bf16)                  # x^T  (e, s)
            wq = big.tile([128, EC, NS], bf16)
            wk = big.tile([128, EC, NS], bf16)
            wv = big.tile([128, EC, NS], bf16)
            wo = big.tile([128, EC, NS], bf16)
            bq_sb = big.tile([128, NT], f32)
            bk_sb = big.tile([128, NT], f32)
            bv_sb = big.tile([128, NT], f32)
            bo_sb = big.tile([128, NT], f32)
            qT = big.tile([128, NT, S], bf16)                  # q^T  (n, s)
            kT = big.tile([128, NT, S], bf16)                  # k^T  (n, t)
            v_sb = big.tile([128, TT, HL, D + 1], bf16)        # v (+ones col)
            aT = big.tile([128, NT, S], bf16)                  # attn out^T (n, s)
            aT_full = big.tile([128, EC, S], bf16)             # gathered attn^T
            po_sb = big.tile([128, NT, S], bf16)               # out-proj pass-1
            ones_sb = big.tile([1, D], bf16)

            for c in range(EC):
                nc.sync.dma_start(xT[:, c, :], x_ext[c])
            nc.sync.dma_start(wq[:], wq_ext[:].rearrange("c p n -> p c n"))
            nc.sync.dma_start(wk[:], wk_ext[:].rearrange("c p n -> p c n"))
            nc.sync.dma_start(wv[:], wv_ext[:].rearrange("c p n -> p c n"))
            nc.sync.dma_start(wo[:], wo_ext[:].rearrange("c p n -> p c n"))
            nc.sync.dma_start(bq_sb[:], bq_ext[:])
            nc.sync.dma_start(bk_sb[:], bk_ext[:])
            nc.sync.dma_start(bv_sb[:], bv_ext[:])
            nc.sync.dma_start(bo_sb[:], bo_ext[:])
            nc.vector.memset(ones_sb[:], 1.0)
            nc.vector.memset(v_sb[:, :, :, D:D + 1], 1.0)

            # ---- background projection generators (one yield per MM) ----
            def gen_qk(w_sb, b_sb, dst, nt, sc):
                ssl = slice(SCW * sc, SCW * (sc + 1))
                ps = ps_w.tile([128, SCW], f32, tag="proj")
                for ec in range(EC):
                    nc.tensor.matmul(
                        ps[:],
                        w_sb[:, ec, 128 * nt:128 * (nt + 1)],
                        xT[:, ec, ssl],
                        start=(ec == 0), stop=(ec == EC - 1),
                    )
                    yield
                nc.vector.tensor_scalar_add(dst[:, nt, ssl], ps[:],
                                            b_sb[:, nt:nt + 1])

            def gen_v(tt):
                ps = ps_w.tile([128, NS], f32, tag="proj")
                for ec in range(EC):
                    nc.tensor.matmul(
                        ps[:],
                        xT[:, ec, 128 * tt:128 * (tt + 1)],
                        wv[:, ec, :],
                        start=(ec == 0), stop=(ec == EC - 1),
                    )
                    yield
                nc.vector.tensor_copy(
                    v_sb[:, tt, :, 0:D],
                    ps[:].rearrange("p (h d) -> p h d", h=HL),
                )

            def bg_chain():
                # kT is a *stationary* operand: emit all of it first so the
                # producing evict always leads the consuming LDWEIGHTS by
                # well over the PE's 64-deep reorder window. qT is a moving
                # operand (streamed at MM execution) -- safe just-in-time.
                for nt in range(1, NT):
                    for sc in range(SC):
                        yield from gen_qk(wk, bk_sb, kT, nt, sc)
                for nt in range(1, NT):
                    for sc in range(SC):
                        yield from gen_qk(wq, bq_sb, qT, nt, sc)

            BG_JIT = os.environ.get("BG_JIT", "1") == "1"
            if BG_JIT:
                bg = bg_chain()
            else:
                bg = iter(())

            def inject(n):
                for _ in range(n):
                    if next(bg, "done") == "done":
                        break

            # ---- preamble: q/k pair 0 + all of v (dense, warms HAM).
            # v is a stationary operand consumed from iteration 0 -- it
            # cannot get a safe lead in the background stream.
            for sc in range(SC):
                for _ in gen_qk(wq, bq_sb, qT, 0, sc):
                    pass
                for _ in gen_qk(wk, bk_sb, kT, 0, sc):
                    pass
            for tt in range(TT):
                for _ in gen_v(tt):
                    pass
            if not BG_JIT:
                for _ in bg_chain():
                    pass

            # ---- attention: ScalarE-paced pipeline over (pair, s-chunk) ----
            def emit_scores_group(p, sc, g):
                """4 score MMs (h0/h64 adjacent pairs) + 2 wide exps."""
                ssl = slice(SCW * sc, SCW * (sc + 1))
                sh = [ps_s.tile([128, 2, SCW], f32, tag="s", name=f"s{i}")
                      for i in range(2)]
                if os.environ.get("SCORE_PAIR", "1") == "1":
                    # h0/h64 adjacent -> concurrent row-group execution
                    for j in range(2):
                        tt = 2 * g + j
                        tsl = slice(128 * tt, 128 * (tt + 1))
                        nc.tensor.matmul(sh[0][:, j, :], kT[0:64, p, tsl],
                                         qT[0:64, p, ssl], start=True, stop=True)
                        nc.tensor.matmul(sh[1][:, j, :], kT[64:128, p, tsl],
                                         qT[64:128, p, ssl], start=True, stop=True)
                else:
                    for i in range(2):
                        o = 64 * i
                        for j in range(2):
                            tt = 2 * g + j
                            tsl = slice(128 * tt, 128 * (tt + 1))
                            nc.tensor.matmul(sh[i][:, j, :],
                                             kT[o:o + 64, p, tsl],
                                             qT[o:o + 64, p, ssl],
                                             start=True, stop=True)
                return sh

            def emit_exps(sh):
                ets = [etp.tile([128, 2, SCW], bf16, tag="et", name=f"et{i}")
                       for i in range(2)]
                for i in range(2):
                    nc.scalar.activation(ets[i][:], sh[i][:], AF.Exp,
                                         scale=float(SCALE))
                return ets

            def emit_attnv_group(p, g, ets, un):
                for j in range(2):
                    tt = 2 * g + j
                    for i in range(2):
                        nc.tensor.matmul(
                            un[0:D + 1, i, :],
                            v_sb[:, tt, 2 * p + i, :],
                            ets[i][:, j, :],
                            start=(tt == 0), stop=(tt == TT - 1),
                        )

            def emit_norm(p, sc, un):
                ssl = slice(SCW * sc, SCW * (sc + 1))
                un_sb = small.tile([128, SCW], f32, tag="un_sb", bufs=1)
                nc.vector.tensor_copy(un_sb[0:D, :], un[0:D, 0, :])
                nc.vector.tensor_copy(un_sb[64:64 + D, :], un[0:D, 1, :])
                denom = small.tile([1, 2, SCW], f32, tag="denom", bufs=1)
                nc.vector.tensor_copy(denom[:], un[D:D + 1, :, :])
                rb = small.tile([1, 2, SCW], f32, tag="rb", bufs=1)
                nc.vector.reciprocal_approx_fast(rb[:], denom[:])
                rb16 = small.tile([1, 2, SCW], bf16, tag="rb16", bufs=1)
                nc.vector.tensor_copy(rb16[:], rb[:])
                bc = ps_w.tile([128, SCW], f32, tag="proj")
                nc.tensor.matmul(bc[0:D, :], ones_sb[:], rb16[0:1, 0, :],
                                 start=True, stop=True)
                nc.tensor.matmul(bc[64:64 + D, :], ones_sb[:], rb16[0:1, 1, :],
                                 start=True, stop=True)
                nc.vector.tensor_mul(aT[:, p, ssl], un_sb[:], bc[:])
                nc.vector.tensor_scalar_add(aT[:, p, ssl], aT[:, p, ssl],
                                            bv_sb[:, p:p + 1])

            def emit_gather(p):
                cc_in = dram.tile([128, 1, S], bf16, name=f"cc_in{p}")
                cc_out = dram.tile([2, 128, 1, S], bf16, name=f"cc_out{p}")
                nc.sync.dma_start(cc_in[:], aT[:, p:p + 1, :])
                nc.gpsimd.collective_compute(
                    "AllGather",
                    mybir.AluOpType.bypass,
                    replica_groups=REPLICA_GROUPS,
                    ins=[cc_in[:].opt()],
                    outs=[cc_out[:].opt()],
                )
                for r in range(2):
                    nc.sync.dma_start(
                        aT_full[:, 2 * p + r:2 * p + r + 1, :], cc_out[r])

            # out-projection pass 1: chunks 0..3 (pairs 0-1, both cores),
            # staged to SBUF bf16; injected into late-attention PE slots.
            def gen_po1():
                for et_i in range(NT):
                    for sc in range(SC):
                        ssl = slice(SCW * sc, SCW * (sc + 1))
                        po = ps_w.tile([128, SCW], f32, tag="proj")
                        for cc in range(4):
                            nc.tensor.matmul(
                                po[:],
                                wo[:, cc, 128 * et_i:128 * (et_i + 1)],
                                aT_full[:, cc, ssl],
                                start=(cc == 0), stop=(cc == 3),
                            )
                            yield
                        nc.vector.tensor_copy(po_sb[:, et_i, ssl], po[:])

            bg2 = gen_po1()

            for p in range(NT):
                for sc in range(SC):
                    it = SC * p + sc
                    un = ps_un.tile([128, 2, SCW], f32, tag="un")
                    prev = None
                    for g in range(NG):
                        sh = emit_scores_group(p, sc, g)
                        if prev is not None:
                            emit_attnv_group(p, g - 1, prev, un)
                        ets = emit_exps(sh)
                        inject(4)
                        if BG_JIT and it >= 9:
                            for _ in range(2):
                                next(bg2, None)
                        prev = ets
                    emit_attnv_group(p, NG - 1, prev, un)
                    emit_norm(p, sc, un)
                    if sc == SC - 1:
                        emit_gather(p)

            for _ in bg2:
                pass

            # ---- out-projection pass 2: chunks 4..7 + staged pass 1 ----
            for et_i in range(NT):
                for sc in range(SC):
                    ssl = slice(SCW * sc, SCW * (sc + 1))
                    po = ps_w.tile([128, SCW], f32, tag="proj")
                    for cc in range(4, EC):
                        nc.tensor.matmul(
                            po[:],
                            wo[:, cc, 128 * et_i:128 * (et_i + 1)],
                            aT_full[:, cc, ssl],
                            start=(cc == 4), stop=(cc == EC - 1),
                        )
                    o_st = small.tile([128, SCW], f32, tag="o_st")
                    # o = (po + bo) + pass1
                    nc.vector.scalar_tensor_tensor(
                        o_st[:], po[:], bo_sb[:, et_i:et_i + 1],
                        po_sb[:, et_i, ssl],
                        mybir.AluOpType.add, mybir.AluOpType.add)
                    nc.sync.dma_start(out_ext[et_i, :, ssl], o_st[:])
    nc.compile()
    return nc


def _prep_inputs(x, Wq, bq, Wk, bk, Wv, bv, Wo, bo):
    """Shard + lay out the full inputs for the 8 cores."""
    import ml_dtypes
    bfl = ml_dtypes.bfloat16

    in_maps = []
    for c in range(N_CORES):
        b, g = divmod(c, G)
        ns = slice(NS * g, NS * (g + 1))
        xT = np.ascontiguousarray(x[b].T).astype(bfl).reshape(EC, 128, S)
        wq_l = np.ascontiguousarray(Wq[ns, :].T).astype(bfl).reshape(EC, 128, NS)
        wk_l = np.ascontiguousarray(Wk[ns, :].T).astype(bfl).reshape(EC, 128, NS)
        wv_l = np.ascontiguousarray(Wv[ns, :].T).astype(bfl).reshape(EC, 128, NS)
        # Wo^T rows (contraction n) in gathered order, cols = this core's
        # e-slice
        woT = np.ascontiguousarray(Wo[ns, :].T)  # [E, NS] = Wo.T[:, es]
        woT = woT.reshape(EC, 128, NS)[WO_ORDER]
        wo_l = woT.astype(bfl).reshape(EC, 128, NS)
        bq_l = np.ascontiguousarray(bq[ns].reshape(NT, 128).T).astype(np.float32)
        bk_l = np.ascontiguousarray(bk[ns].reshape(NT, 128).T).astype(np.float32)
        bv_l = np.ascontiguousarray(bv[ns].reshape(NT, 128).T).astype(np.float32)
        bo_l = np.ascontiguousarray(bo[ns].reshape(NT, 128).T).astype(np.float32)
        in_maps.append({
            "xT": np.ascontiguousarray(xT),
            "wq": np.ascontiguousarray(wq_l),
            "wk": np.ascontiguousarray(wk_l),
            "wv": np.ascontiguousarray(wv_l),
            "wo": np.ascontiguousarray(wo_l),
            "bq": bq_l, "bk": bk_l, "bv": bv_l, "bo": bo_l,
        })
    return in_maps


def kernel(x, Wq, bq, Wk, bk, Wv, bv, Wo, bo, _trace=False):
    x = np.asarray(x)
    in_maps = _prep_inputs(np.asarray(x, np.float32),
                           *[np.asarray(a, np.float32)
                             for a in (Wq, bq, Wk, bk, Wv, bv, Wo, bo)])
    if "nc" not in _CACHE:
        _CACHE["nc"] = build()
    nc = _CACHE["nc"]
    res = run_bass_kernel_spmd(nc, in_maps, core_ids=list(range(N_CORES)),
                               trace=_trace)
    _CACHE["last_result"] = res

    out = np.empty((B, S, E), np.float32)
    for c in range(N_CORES):
        b, g = divmod(c, G)
        oT = res.results[c]["out"].reshape(NS, S)  # [e_sub, s]
        out[b, :, NS * g:NS * (g + 1)] = oT.T
    return out


if __name__ == "__main__":
    nc = build()
    print("built ok:", len(nc.inst_map), "instructions")
